# revision 17
# baseline (speedup 1.0000x reference)
"""AttentionPooling (segment softmax + weighted segment sum) on 8 trn2 cores.

Strategy: shard whole segments across cores (sorted batch -> contiguous node
ranges).  Host pre-casts x to bf16 and uploads BOTH orientations per core as
chunk-major contiguous blocks (two tensors -> two concurrent DMA streams):
  xn [c, p, t, 257]   node-partitioned (col 256 = ones, yields denominators)
  xt [c, p, 2, 4096]  channel-partitioned (for the score matmul)
so the device reads 64MB of contiguous bf16 per core and does no cast,
bounce, or transpose on chip.  Per 4096-node chunk: PE computes
h = tanh(xT @ W1 + b1) (hidden-partitioned), per-tile score columns
s = h_tile.T @ W2, ACT exponentiates, DVE builds we = onehot(batch)*e, and
PE accumulates [64,257] = we.T @ [x | 1] in PSUM across all chunks
(column 256 = softmax denominators).  wsum(c-1) is emitted between h(c) and
score(c) so PE never stalls on the tanh tail.  The tail chunk is partial
(variable tile count) so node padding is <1%.  Softmax max-subtraction is
skipped: |s| <= ||W2||_1 + |b2| ~ 28, exp stays in fp32 range.
"""

from contextlib import ExitStack

import numpy as np
import ml_dtypes

import concourse.bass as bass
import concourse.bacc as bacc
import concourse.tile as tile
from concourse import mybir
from concourse.bass_utils import run_bass_kernel_spmd

N_CORES = 8
NUM_GRAPHS = 512
SEGS_PER_CORE = NUM_GRAPHS // N_CORES  # 64
D = 256          # in channels
H = 128          # hidden
P = 128          # partitions
TILE_N = 128     # nodes per weight tile
CHUNK_T = 32     # max tiles per chunk
CHUNK_N = TILE_N * CHUNK_T  # 4096 nodes per full chunk
DW = D + 1       # node-partitioned row width: 256 channels + ones column

_BF16 = mybir.dt.bfloat16
_F32 = mybir.dt.float32
_I32 = mybir.dt.int32


def _chunk_schedule(n_tiles: int):
    """Small chunks at both ends (fast pipeline fill / short drain), 32-tile
    chunks in the middle, remainder as one partial chunk before the tail."""
    lead, tail = [8, 16], [16, 8, 8]
    if n_tiles <= sum(lead) + sum(tail) + CHUNK_T:
        chunks = [CHUNK_T] * (n_tiles // CHUNK_T)
        if n_tiles % CHUNK_T:
            chunks.append(n_tiles % CHUNK_T)
        return chunks
    mid = n_tiles - sum(lead) - sum(tail)
    chunks = lead + [CHUNK_T] * (mid // CHUNK_T)
    if mid % CHUNK_T:
        chunks.append(mid % CHUNK_T)
    return chunks + tail


def _build_program(n_tiles: int, b2_val: float):
    nc = bacc.Bacc()
    chunks = _chunk_schedule(n_tiles)
    n_chunks = len(chunks)

    xn_ds = [nc.declare_dram_parameter(f"xn{c}", [P, tc * DW], _BF16,
                                       isOutput=False)
             for c, tc in enumerate(chunks)]
    xt_ds = [nc.declare_dram_parameter(f"xt{c}", [P, 2 * tc * TILE_N], _BF16,
                                       isOutput=False)
             for c, tc in enumerate(chunks)]
    bt_d = nc.declare_dram_parameter("batch_t", [P, n_tiles + SEGS_PER_CORE],
                                     _I32, isOutput=False)
    w1_d = nc.declare_dram_parameter("w1", [D, H], _BF16, isOutput=False)
    w2_d = nc.declare_dram_parameter("w2", [H, 1], _BF16, isOutput=False)
    b1_d = nc.declare_dram_parameter("b1", [H, 1], _F32, isOutput=False)
    out_d = nc.declare_dram_parameter("out_g", [SEGS_PER_CORE, D], _F32,
                                      isOutput=True)

    xn_aps = [d[:].rearrange("p (t w) -> p t w", w=DW) for d in xn_ds]
    xt_aps = [d[:].rearrange("p (h n) -> p h n", h=2) for d in xt_ds]

    with tile.TileContext(nc) as tc, ExitStack() as ctx:
        const_pool = ctx.enter_context(tc.tile_pool(name="consts", bufs=1))
        xn_pool = ctx.enter_context(tc.tile_pool(name="xn", bufs=4))
        xt_pool = ctx.enter_context(tc.tile_pool(name="xt", bufs=4))
        h_pool = ctx.enter_context(tc.tile_pool(name="h", bufs=2))
        cmp_pool = ctx.enter_context(tc.tile_pool(name="cmp", bufs=2))
        we_pool = ctx.enter_context(tc.tile_pool(name="we", bufs=2))
        ecol_pool = ctx.enter_context(tc.tile_pool(name="ecol", bufs=2))
        fin_pool = ctx.enter_context(tc.tile_pool(name="fin", bufs=1))
        psum_h = ctx.enter_context(
            tc.tile_pool(name="psum_h", bufs=2, space=bass.MemorySpace.PSUM))
        psum_s = ctx.enter_context(
            tc.tile_pool(name="psum_s", bufs=2, space=bass.MemorySpace.PSUM))
        psum_acc = ctx.enter_context(
            tc.tile_pool(name="psum_acc", bufs=1, space=bass.MemorySpace.PSUM))

        # ---- constants / weights ----
        w1_sb = const_pool.tile([P, 2, H], _BF16, tag="w1")   # [:, 0, :]=ch 0-127
        nc.sync.dma_start(w1_sb[:, 0, :], w1_d[0:128, :])
        nc.sync.dma_start(w1_sb[:, 1, :], w1_d[128:256, :])
        w2_sb = const_pool.tile([P, 1], _BF16, tag="w2")
        nc.sync.dma_start(w2_sb[:], w2_d[:])
        b1_sb = const_pool.tile([P, 1], _F32, tag="b1")
        nc.sync.dma_start(b1_sb[:], b1_d[:])
        bt_sb = const_pool.tile([P, n_tiles + SEGS_PER_CORE], _I32, tag="bt")
        nc.sync.dma_start(bt_sb[:], bt_d[:])
        iota_sb = bt_sb[:, n_tiles:n_tiles + SEGS_PER_CORE]

        acc_ps = psum_acc.tile([SEGS_PER_CORE, DW], _F32, tag="acc")

        saved = {}

        def emit_load_h(c, t0, tc_):
            xn_t = xn_pool.tile([P, CHUNK_T, DW], _BF16, tag="xn")
            xt_t = xt_pool.tile([P, 2, CHUNK_N], _BF16, tag="xt")
            nc.sync.dma_start(xn_t[:, 0:tc_, :], xn_aps[c])
            nc.scalar.dma_start(xt_t[:, :, 0:tc_ * TILE_N], xt_aps[c])

            # h = tanh(x @ W1 + b1), hidden-partitioned, bf16
            h_bf = h_pool.tile([P, CHUNK_N], _BF16, tag="h")
            for s0 in range(0, tc_ * TILE_N, 512):
                sn = min(512, tc_ * TILE_N - s0)
                ph = psum_h.tile([P, 512], _F32, tag="ph")
                nc.tensor.matmul(ph[:, 0:sn], w1_sb[:, 0, :],
                                 xt_t[:, 0, s0:s0 + sn],
                                 start=True, stop=False)
                nc.tensor.matmul(ph[:, 0:sn], w1_sb[:, 1, :],
                                 xt_t[:, 1, s0:s0 + sn],
                                 start=False, stop=True)
                nc.scalar.activation(h_bf[:, s0:s0 + sn], ph[:, 0:sn],
                                     mybir.ActivationFunctionType.Tanh,
                                     bias=b1_sb[:])
            saved[c] = (xn_t, h_bf)

        def emit_score(c, t0, tc_):
            xn_t, h_bf = saved[c]
            # per-tile score columns: s_col[p, t] = h_tile.T @ W2
            ps_s = psum_s.tile([P, CHUNK_T], _F32, tag="ps_s")
            for t in range(tc_):
                nc.tensor.matmul(ps_s[:, t:t + 1],
                                 h_bf[:, t * TILE_N:(t + 1) * TILE_N],
                                 w2_sb, start=True, stop=True)

            # e = exp(s + b2)  (node-partitioned, bf16)
            e_col = ecol_pool.tile([P, CHUNK_T], _BF16, tag="ecol")
            nc.scalar.activation(e_col[:, 0:tc_], ps_s[:, 0:tc_],
                                 mybir.ActivationFunctionType.Exp,
                                 bias=float(b2_val))

            # we[p, t, g] = (batch_t == g) * e   (bf16)
            cmp = cmp_pool.tile([P, CHUNK_T, SEGS_PER_CORE], _BF16, tag="cmp")
            bt_c = bt_sb[:, t0:t0 + tc_]
            nc.vector.tensor_tensor(
                cmp[:, 0:tc_],
                bt_c.unsqueeze(2).broadcast_to([P, tc_, SEGS_PER_CORE]),
                iota_sb.unsqueeze(1).broadcast_to([P, tc_, SEGS_PER_CORE]),
                mybir.AluOpType.is_equal)
            we = we_pool.tile([P, CHUNK_T, SEGS_PER_CORE], _BF16, tag="we")
            nc.vector.tensor_tensor(
                we[:, 0:tc_], cmp[:, 0:tc_],
                e_col[:, 0:tc_].unsqueeze(2).broadcast_to(
                    [P, tc_, SEGS_PER_CORE]),
                mybir.AluOpType.mult)
            saved[c] = (xn_t, we)

        def emit_wsum(c, tc_, first, last):
            xn_t, we = saved.pop(c)
            for t in range(tc_):
                nc.tensor.matmul(acc_ps[:], we[:, t, :], xn_t[:, t, :],
                                 start=(first and t == 0),
                                 stop=(last and t == tc_ - 1),
                                 skip_group_check=True)

        t0s = np.concatenate([[0], np.cumsum(chunks)]).astype(int)
        for c in range(n_chunks):
            emit_load_h(c, int(t0s[c]), chunks[c])
            if c >= 1:
                emit_wsum(c - 1, chunks[c - 1], c - 1 == 0, False)
            emit_score(c, int(t0s[c]), chunks[c])
        emit_wsum(n_chunks - 1, chunks[-1], n_chunks == 1, True)

        # ---- epilogue: out = acc[:, :256] / acc[:, 256] ----
        den_sb = fin_pool.tile([SEGS_PER_CORE, 1], _F32, tag="den_sb")
        nc.vector.tensor_scalar_add(den_sb[:], acc_ps[:, D:DW], 1e-30)
        rec_sb = fin_pool.tile([SEGS_PER_CORE, 1], _F32, tag="rec_sb")
        nc.vector.reciprocal(rec_sb[:], den_sb[:])
        out_sb = fin_pool.tile([SEGS_PER_CORE, D], _F32, tag="out_sb")
        nc.vector.tensor_scalar_mul(out_sb[:], acc_ps[:, 0:D], rec_sb[:])
        nc.sync.dma_start(out_d[:], out_sb[:])

    return nc


def _prepare_inputs(x, W1, b1, W2, b2, batch):
    x = np.asarray(x)
    batch = np.asarray(batch).astype(np.int64)
    # core k owns segments [64k, 64(k+1)); sorted batch -> contiguous ranges
    bounds = np.searchsorted(batch, np.arange(0, NUM_GRAPHS + 1, SEGS_PER_CORE))
    counts = np.diff(bounds)
    nmax = int(np.max(counts))
    n_tiles = max(1, (nmax + TILE_N - 1) // TILE_N)
    nmax_pad = n_tiles * TILE_N
    chunks = _chunk_schedule(n_tiles)
    t0s = np.concatenate([[0], np.cumsum(chunks)]).astype(int)

    x_bf = x.astype(ml_dtypes.bfloat16)
    w1_bf = np.asarray(W1, np.float32).astype(ml_dtypes.bfloat16)
    w2_bf = np.asarray(W2, np.float32).reshape(H, 1).astype(ml_dtypes.bfloat16)
    b1_col = np.asarray(b1, np.float32).reshape(H, 1)

    in_maps = []
    for k in range(N_CORES):
        lo, hi = int(bounds[k]), int(bounds[k + 1])
        cnt = hi - lo
        xp = np.zeros((nmax_pad, D), ml_dtypes.bfloat16)
        xp[:cnt] = x_bf[lo:hi]
        # node-partitioned [t, p, ch] + ones column
        xn = np.empty((n_tiles, P, DW), ml_dtypes.bfloat16)
        xn[:, :, 0:D] = xp.reshape(n_tiles, P, D)
        xn[:, :, D] = 1.0
        # channel-partitioned [t, p(ch in half), half, node-in-tile]
        xt = np.ascontiguousarray(
            xp.reshape(n_tiles, TILE_N, 2, P).transpose(0, 3, 2, 1))

        m = {"w1": w1_bf, "w2": w2_bf, "b1": b1_col}
        for c, tc in enumerate(chunks):
            a, b = int(t0s[c]), int(t0s[c + 1])
            m[f"xn{c}"] = np.ascontiguousarray(
                xn[a:b].transpose(1, 0, 2)).reshape(P, tc * DW)
            # [p, half, node-in-chunk]
            m[f"xt{c}"] = np.ascontiguousarray(
                xt[a:b].transpose(1, 2, 0, 3)).reshape(P, 2 * tc * TILE_N)

        bt = np.full((nmax_pad,), -1, np.int32)
        bt[:cnt] = batch[lo:hi] - k * SEGS_PER_CORE
        bt_t = bt.reshape(n_tiles, P).T  # (128, n_tiles)
        iota_cols = np.tile(np.arange(SEGS_PER_CORE, dtype=np.int32), (P, 1))
        bt_t = np.concatenate([bt_t, iota_cols], axis=1).copy()
        m["batch_t"] = bt_t
        in_maps.append(m)
    return in_maps, n_tiles


def run(x, W1, b1, W2, b2, batch, trace=False, trace_kwargs=None):
    in_maps, n_tiles = _prepare_inputs(x, W1, b1, W2, b2, batch)
    nc = _build_program(n_tiles, float(np.asarray(b2).reshape(-1)[0]))
    nc.finalize()
    res = run_bass_kernel_spmd(nc, in_maps, list(range(N_CORES)),
                               trace=trace, **(trace_kwargs or {}))
    out = np.concatenate([np.asarray(res.results[k]["out_g"], np.float32)
                          for k in range(N_CORES)], axis=0)
    return out, res


def kernel(x, W1, b1, W2, b2, batch):
    out, _ = run(x, W1, b1, W2, b2, batch)
    return out
